# revision 12
# baseline (speedup 1.0000x reference)
"""ASGFormer (graph-transformer message passing) Trainium2 kernel.

Strategy (8 NeuronCores, SPMD):
  * Nodes are sharded by destination: core c owns nodes [c*1280, (c+1)*1280)
    (N=10000 padded to 10240). Edges are sorted by dst on the host so every
    edge lands on the core that owns its target node; the segment softmax and
    the scatter-add aggregation become core-local (no collectives at all).
  * Per 128-node group, edges are processed in 128-edge chunks. A 0/1
    one-hot [edge, local-node] matrix (built with iota + is_equal) turns both
    the "expand Q to edges" gather and the "scatter-add to nodes" reduction
    into TensorEngine matmuls that accumulate in PSUM.
  * K|V|pos rows are gathered per edge from a precomputed DRAM table with
    the hardware row-gather DMA (dma_gather).
  * The softmax max-subtraction is dropped (mathematically a no-op for
    softmax; scores here are O(1) so exp cannot overflow), and the
    positional-embedding LayerNorm is folded into the score:
        q . (k + LN(y)*g + b) = q.k + qb + rs*( (q*g).y - mu*sum(q*g) )
    with qb/qgs precomputed per node, so pe is never materialized.
  * fp32 everywhere except the bulk per-edge matmuls which use float32r
    (TF32-like, ~1e-4 relative) for 4x TensorEngine throughput. The final
    edge->node aggregation matmul stays fp32.
"""
import numpy as np
from contextlib import ExitStack

import concourse.bass as bass
import concourse.tile as tile
from concourse import bacc, mybir
from concourse.bass_utils import run_bass_kernel_spmd

f32 = mybir.dt.float32
f32r = mybir.dt.float32r
i16 = mybir.dt.int16
i32 = mybir.dt.int32
AF = mybir.ActivationFunctionType
OP = mybir.AluOpType

N, E, D, H = 10000, 160000, 256, 8
HD = D // H
NCORES = 8
NPC = 1280            # nodes per core (padded)
GPC = 10              # 128-node groups per core
NPAD = NCORES * NPC   # 10240
GCALL = 8             # chunks per kv-gather call
KVW = 576             # kv row: K(256) | V(256) | pos(3) | pad -> 2304B


# ----------------------------------------------------------------------
# host-side index prep (pure index manipulation, no float math)
# ----------------------------------------------------------------------
def _prep(edge_index):
    src = np.asarray(edge_index[0]).astype(np.int64)
    dst = np.asarray(edge_index[1]).astype(np.int64)
    order = np.argsort(dst, kind="stable")
    src_s, dst_s = src[order], dst[order]
    gid = dst_s // 128
    counts = np.bincount(gid, minlength=NCORES * GPC)
    padg = int(np.ceil(max(counts.max(), 1) / 128) * 128)
    ch = padg // 128                       # chunks per group
    nch = GPC * ch                         # chunks per core
    ncalls = int(np.ceil(nch / GCALL))
    nch_pad = ncalls * GCALL
    pos_ = np.concatenate([[0], np.cumsum(counts)])
    per_core = []
    for c in range(NCORES):
        srcs = np.zeros(nch_pad * 128, np.int64)
        dstr = -np.ones(nch_pad * 128, np.float32)
        for g in range(GPC):
            gg = c * GPC + g
            s, e = pos_[gg], pos_[gg + 1]
            srcs[g * padg:g * padg + (e - s)] = src_s[s:e]
            dstr[g * padg:g * padg + (e - s)] = dst_s[s:e] - gg * 128
        # kv gather indices, wrapped over 16 partitions: idx j -> [j%16, j//16]
        wrapped = srcs.reshape(-1, 16).T.astype(np.int16)
        kvidx = np.tile(wrapped, (8, 1))
        # dstrel per chunk column: [lane, chunk]
        dstrel = dstr.reshape(nch_pad, 128).T.copy()
        per_core.append((kvidx, dstrel))
    return per_core, padg, ch, nch, ncalls


# ----------------------------------------------------------------------
# device program
# ----------------------------------------------------------------------
def _build(ch, nch, ncalls, reps=1, debug=False):
    nch_pad = ncalls * GCALL
    nc = bacc.Bacc("TRN2", target_bir_lowering=False, debug=False,
                   num_devices=NCORES)
    DT = lambda n, s, d=f32, k="ExternalInput": nc.dram_tensor(n, s, d, kind=k).ap()

    xT = DT("xT", [256, NPAD])            # full x, transposed (replicated)
    posf = DT("posf", [NPAD, 4])          # full pos padded (replicated)
    xTl = DT("xTl", [256, NPC])           # local x transposed
    xl = DT("xl", [NPC, 256])             # local x (residual)
    posl = DT("posl", [NPC, 4])           # local pos padded
    kvidx = DT("kvidx", [128, nch_pad * 8], i16)
    dstrel = DT("dstrel", [128, nch_pad])
    wkv = DT("wkv", [256, 512])           # [Wk | Wv]
    bkv = DT("bkv", [1, 512])             # [bk + bp_beta | bv]
    wq = DT("wq", [256, 256])
    bq_r = DT("bq_r", [1, 256])
    gp_b = DT("gp_b", [128, 256])         # gamma_p broadcast
    bp_b = DT("bp_b", [128, 256])         # beta_p broadcast
    wp1 = DT("wp1", [4, 256])
    bp1c = DT("bp1c", [256, 1])
    wp2 = DT("wp2", [256, 256])
    bp2_r = DT("bp2_r", [1, 256])
    wo = DT("wo", [256, 256])
    bo_r = DT("bo_r", [1, 256])
    g1_b = DT("g1_b", [128, 256])
    b1_b = DT("b1_b", [128, 256])
    wf1 = DT("wf1", [256, 1024])
    bf1_r = DT("bf1_r", [1, 1024])
    wf2 = DT("wf2", [1024, 256])
    bf2_r = DT("bf2_r", [1, 256])
    g2_b = DT("g2_b", [128, 256])
    b2_b = DT("b2_b", [128, 256])
    out_d = DT("out", [NPC, 256], f32, "ExternalOutput")
    dbg = {}
    if debug:
        for nm, sh in [("d_kvv", [128, KVW]), ("d_ohe", [128, 128]),
                       ("d_dpt", [4, 128]), ("d_h1s", [128, 256]),
                       ("d_ysb", [128, 256]), ("d_qx1", [128, 512]),
                       ("d_red", [128, 16]), ("d_s3", [128, 8]),
                       ("d_esw", [128, 264]), ("d_aggr", [128, 264]),
                       ("d_mvrs", [128, 3]), ("d_qx", [128, 528])]:
            dbg[nm] = DT(nm, sh, f32, "ExternalOutput")
    kv_dram = nc.dram_tensor("kv_dram", [NPAD, KVW], f32, kind="Internal").ap()

    SQH = float(1.0 / np.sqrt(HD))

    with tile.TileContext(nc) as tc, ExitStack() as ctx:
        cpool = ctx.enter_context(tc.tile_pool(name="consts", bufs=1))
        wpool = ctx.enter_context(tc.tile_pool(name="weights", bufs=1))
        qxpool = ctx.enter_context(tc.tile_pool(name="qx", bufs=1))
        xpool = ctx.enter_context(tc.tile_pool(name="xslab", bufs=2))
        kvpool = ctx.enter_context(tc.tile_pool(name="kvg", bufs=2))
        epool = ctx.enter_context(tc.tile_pool(name="edge", bufs=2))
        spool = ctx.enter_context(tc.tile_pool(name="small", bufs=2))
        npool = ctx.enter_context(tc.tile_pool(name="node", bufs=2))
        # PSUM: exactly 8 banks, all single-buffered
        pp = ctx.enter_context(tc.tile_pool(name="ps", bufs=1, space="PSUM"))

        # ---- constants ----
        iota_i = cpool.tile([128, 128], i32)
        nc.gpsimd.iota(iota_i[:], pattern=[[1, 128]], base=0, channel_multiplier=0)
        iota_f = cpool.tile([128, 128], f32)
        nc.vector.tensor_copy(iota_f[:], iota_i[:])
        pidx_i = cpool.tile([128, 1], i32)
        nc.gpsimd.iota(pidx_i[:], pattern=[[0, 1]], base=0, channel_multiplier=1)
        pidx_f = cpool.tile([128, 1], f32)
        nc.vector.tensor_copy(pidx_f[:], pidx_i[:])
        ident = cpool.tile([128, 128], f32)
        nc.vector.tensor_scalar(ident[:], iota_f[:], pidx_f[:], None, op0=OP.is_equal)
        nident = cpool.tile([128, 128], f32)
        nc.vector.tensor_scalar_mul(nident[:], ident[:], -1.0)
        ones1f = cpool.tile([1, 128], f32)
        nc.vector.memset(ones1f[:], 1.0)
        ones1r = cpool.tile([1, 128], f32r)
        nc.vector.tensor_copy(ones1r[:], ones1f[:])
        epst = cpool.tile([128, 1], f32)
        nc.vector.memset(epst[:], 1e-5)
        eps16 = cpool.tile([128, 1], f32)
        nc.vector.memset(eps16[:], 1e-16)
        zpad = cpool.tile([128, 64], f32)
        nc.vector.memset(zpad[:], 0.0)

        # ---- weights: load (+ cast to f32r where used in f32r matmuls) ----
        def ldw(name, dram_ap, shape, dt):
            t = wpool.tile(shape, dt, tag=name)
            nc.gpsimd.dma_start(t[:], dram_ap)
            return t
        wkv_r = ldw("wkv_r", wkv.rearrange("(k p) c -> p k c", p=128),
                    [128, 2, 512], f32r)
        bkv_r = ldw("bkv_r", bkv, [1, 512], f32r)
        wq_r = ldw("wq_r", wq.rearrange("(k p) c -> p k c", p=128),
                   [128, 2, 256], f32r)
        bq_rr = ldw("bq_rr", bq_r, [1, 256], f32r)
        wp1_f = ldw("wp1_f", wp1, [4, 256], f32)
        wp1_r = cpool.tile([4, 256], f32r)
        nc.vector.tensor_scalar_mul(wp1_r[:], wp1_f[:], -1.0)
        wp2_r = ldw("wp2_r", wp2.rearrange("(k p) c -> p k c", p=128),
                    [128, 2, 256], f32r)
        bp2_rr = ldw("bp2_rr", bp2_r, [1, 256], f32r)
        wo_r = ldw("wo_r", wo.rearrange("(k p) c -> p k c", p=128),
                   [128, 2, 256], f32r)
        bo_rr = ldw("bo_rr", bo_r, [1, 256], f32r)
        wf1_r = ldw("wf1_r", wf1.rearrange("(k p) c -> p k c", p=128),
                    [128, 2, 1024], f32r)
        bf1_rr = ldw("bf1_rr", bf1_r, [1, 1024], f32r)
        wf2_r = ldw("wf2_r", wf2.rearrange("(s p) c -> p s c", p=128),
                    [128, 8, 256], f32r)
        bf2_rr = ldw("bf2_rr", bf2_r, [1, 256], f32r)
        bp1_t = ldw("bp1_t", bp1c.rearrange("(k p) one -> p (k one)", p=128),
                    [128, 2], f32)
        gp_t = ldw("gp_t", gp_b, [128, 256], f32)
        bp_t = ldw("bp_t", bp_b, [128, 256], f32)
        g1_t = ldw("g1_t", g1_b, [128, 256], f32)
        b1_t = ldw("b1_t", b1_b, [128, 256], f32)
        g2_t = ldw("g2_t", g2_b, [128, 256], f32)
        b2_t = ldw("b2_t", b2_b, [128, 256], f32)
        kvix = ldw("kvix", kvidx, [128, nch_pad * 8], i16)
        drel = ldw("drel", dstrel, [128, nch_pad], f32)
        posl_f = ldw("posl_f", posl.rearrange("(g p) c -> p g c", p=128),
                     [128, GPC, 4], f32)
        posl_r = cpool.tile([128, GPC, 4], f32r)
        nc.vector.tensor_scalar_mul(posl_r[:], posl_f[:], -1.0)

        def bias_mm(ps_ap, row_r, start=False, stop=False):
            nc.tensor.matmul(ps_ap, ones1r[:, 0:ps_ap.shape[0]], row_r,
                             start=start, stop=stop)

        with nc.allow_low_precision("f32r intermediate stages by design"):
          for rep in range(reps):
            # ============ phase 1: KV table (replicated full graph) ============
            for sl in range(8):
                xs0 = xpool.tile([128, 1280], f32r, tag="xs0")
                nc.gpsimd.dma_start(xs0[:], xT[0:128, sl * 1280:(sl + 1) * 1280])
                xs1 = xpool.tile([128, 1280], f32r, tag="xs1")
                nc.gpsimd.dma_start(xs1[:], xT[128:256, sl * 1280:(sl + 1) * 1280])
                for t in range(10):
                    nt = sl * 10 + t
                    kvp = pp.tile([128, 512], f32, tag="bank_a")
                    nc.tensor.matmul(kvp[:], xs0[:, t * 128:(t + 1) * 128],
                                     wkv_r[:, 0, :], start=True, stop=False)
                    nc.tensor.matmul(kvp[:], xs1[:, t * 128:(t + 1) * 128],
                                     wkv_r[:, 1, :], start=False, stop=False)
                    bias_mm(kvp[:], bkv_r[:], stop=True)
                    kvs = npool.tile([128, 512], f32, tag="kvs")
                    nc.scalar.copy(kvs[:], kvp[:])
                    nc.sync.dma_start(
                        kv_dram[nt * 128:(nt + 1) * 128, 0:512], kvs[:])
                    nc.sync.dma_start(
                        kv_dram[nt * 128:(nt + 1) * 128, 515:576],
                        zpad[:, 0:61])
            nc.sync.dma_start(kv_dram[:, 512:515], posf[:, 0:3])

            # ============ phase 2: Q-side per group ============
            xlT = wpool.tile([128, 2, NPC], f32r, tag="xlT")
            nc.gpsimd.dma_start(xlT[:], xTl.rearrange("(k p) c -> p k c", p=128))
            qxs = []
            for g in range(GPC):
                qp = pp.tile([128, 256], f32, tag="bank_a")
                nc.tensor.matmul(qp[:], xlT[:, 0, g * 128:(g + 1) * 128],
                                 wq_r[:, 0, :], start=True, stop=False)
                nc.tensor.matmul(qp[:], xlT[:, 1, g * 128:(g + 1) * 128],
                                 wq_r[:, 1, :], start=False, stop=False)
                bias_mm(qp[:], bq_rr[:], stop=True)
                qx = qxpool.tile([128, 528], f32r, tag=f"qx{g}")
                nc.scalar.copy(qx[:, 0:256], qp[:])
                nc.vector.tensor_mul(qx[:, 256:512], qp[:], gp_t[:])
                qbt = npool.tile([128, 256], f32, tag="qbt")
                nc.vector.tensor_mul(qbt[:], qp[:], bp_t[:])
                nc.vector.tensor_reduce(
                    qx[:, 512:520].rearrange("p (h one) -> p h one", one=1),
                    qbt[:].rearrange("p (h w) -> p h w", h=H),
                    axis=mybir.AxisListType.X, op=OP.add)
                nc.vector.tensor_reduce(
                    qx[:, 520:528].rearrange("p (h one) -> p h one", one=1),
                    qx[:, 256:512].rearrange("p (h w) -> p h w", h=H),
                    axis=mybir.AxisListType.X, op=OP.add)
                qxs.append(qx)

            # ============ phase 3: edge chunks ============
            # PSUM banks: bank_a(misc: ohp|h1p|dpp), yq, qx1, aggr,
            #             nmisc(agp|op_/x1p), o2p, fp, fqp  -> 8 total
            kvts = {}
            for g in range(GPC):
                aggr = pp.tile([128, 264], f32, tag="aggr")
                for c_ in range(ch):
                    j = g * ch + c_
                    call = j // GCALL
                    if call not in kvts:
                        kvt = kvpool.tile([128, GCALL, KVW], f32, tag="kvt")
                        nc.gpsimd.dma_gather(
                            kvt[:], kv_dram[:, :],
                            kvix[:, call * GCALL * 8:(call + 1) * GCALL * 8],
                            num_idxs=GCALL * 128, num_idxs_reg=GCALL * 128,
                            elem_size=KVW)
                        kvts = {call: kvt}
                    kvv = kvts[call][:, j % GCALL, :]

                    misc = pp.tile([128, 512], f32, tag="bank_a")
                    ohp = misc[:, 0:128]
                    h1p = misc[:, 128:384]
                    dpp = misc[0:4, 384:512]

                    ohe = epool.tile([128, 128], f32, tag="ohe")
                    nc.vector.tensor_scalar(ohe[:], iota_f[:],
                                            drel[:, j:j + 1], None,
                                            op0=OP.is_equal)
                    nc.tensor.transpose(ohp, ohe[:], ident[:])
                    ohn = epool.tile([128, 128], f32r, tag="ohn")
                    nc.scalar.copy(ohn[:], ohp)

                    nc.tensor.matmul(dpp[0:4, :], kvv[:, 512:516], ident[:],
                                     is_transpose=True, start=True, stop=False)
                    nc.tensor.matmul(dpp, posl_r[:, g, :], ohn[:],
                                     start=False, stop=True)
                    dpt = spool.tile([4, 128], f32r, tag="dpt")
                    nc.scalar.copy(dpt[:], dpp)

                    nc.tensor.matmul(h1p[:, 0:128], wp1_r[:, 0:128], dpt[:],
                                     start=True, stop=True)
                    nc.tensor.matmul(h1p[:, 128:256], wp1_r[:, 128:256],
                                     dpt[:], start=True, stop=True)
                    h1s = epool.tile([128, 256], f32r, tag="h1s")
                    nc.scalar.activation(h1s[:, 0:128], h1p[:, 0:128], AF.Relu,
                                         bias=bp1_t[:, 0:1])
                    nc.scalar.activation(h1s[:, 128:256], h1p[:, 128:256],
                                         AF.Relu, bias=bp1_t[:, 1:2])

                    yq = pp.tile([128, 272], f32, tag="yq")
                    y = yq[:, 0:256]
                    nc.tensor.matmul(y, h1s[:, 0:128], wp2_r[:, 0, :],
                                     start=True, stop=False)
                    nc.tensor.matmul(y, h1s[:, 128:256], wp2_r[:, 1, :],
                                     start=False, stop=False)
                    bias_mm(y, bp2_rr[:], stop=True)
                    nc.tensor.matmul(yq[:, 256:272], ohn[:],
                                     qxs[g][:, 512:528], start=True, stop=True)
                    qx1 = pp.tile([128, 512], f32, tag="qx1")
                    nc.tensor.matmul(qx1[:], ohn[:], qxs[g][:, 0:512],
                                     start=True, stop=True)

                    ysb = epool.tile([128, 256], f32, tag="ysb")
                    nc.scalar.copy(ysb[:], y)
                    st6 = spool.tile([128, 6], f32, tag="st6")
                    nc.vector.bn_stats(st6[:], ysb[:])
                    mv = spool.tile([128, 2], f32, tag="mv")
                    nc.vector.bn_aggr(mv[:], st6[:])
                    lnv = spool.tile([128, 1], f32, tag="lnv")
                    nc.scalar.activation(lnv[:], mv[:, 1:2], AF.Ln, bias=epst[:])
                    rs = spool.tile([128, 1], f32, tag="rs")
                    nc.scalar.activation(rs[:], lnv[:], AF.Exp, scale=-0.5)

                    u2 = epool.tile([128, 512], f32, tag="u2")
                    nc.vector.tensor_mul(u2[:, 0:256], qx1[:, 0:256],
                                         kvv[:, 0:256])
                    nc.vector.tensor_mul(u2[:, 256:512], qx1[:, 256:512],
                                         ysb[:])
                    red = spool.tile([128, 16], f32, tag="red")
                    nc.vector.tensor_reduce(
                        red[:].rearrange("p (h one) -> p h one", one=1),
                        u2[:].rearrange("p (h w) -> p h w", h=16),
                        axis=mybir.AxisListType.X, op=OP.add)
                    t2 = spool.tile([128, 8], f32, tag="t2")
                    nc.vector.scalar_tensor_tensor(
                        t2[:], yq[:, 264:272], mv[:, 0:1], red[:, 8:16],
                        op0=OP.mult, op1=OP.subtract)
                    s2 = spool.tile([128, 8], f32, tag="s2")
                    nc.vector.scalar_tensor_tensor(
                        s2[:], t2[:], rs[:], red[:, 0:8],
                        op0=OP.mult, op1=OP.subtract)
                    s3 = spool.tile([128, 8], f32, tag="s3")
                    nc.vector.scalar_tensor_tensor(
                        s3[:], yq[:, 256:264], 0.0, s2[:],
                        op0=OP.bypass, op1=OP.subtract)
                    if debug and j == 0:
                        nc.sync.dma_start(dbg["d_kvv"], kvv)
                        nc.sync.dma_start(dbg["d_ohe"], ohe[:])
                        dd = npool.tile([4, 128], f32, tag="dd")
                        nc.vector.tensor_copy(dd[:], dpt[:].bitcast(f32))
                        nc.sync.dma_start(dbg["d_dpt"], dd[:])
                        dh = npool.tile([128, 256], f32, tag="dh")
                        nc.vector.tensor_copy(dh[:], h1s[:].bitcast(f32))
                        nc.sync.dma_start(dbg["d_h1s"], dh[:])
                        nc.sync.dma_start(dbg["d_ysb"], ysb[:])
                        dq = npool.tile([128, 512], f32, tag="dq")
                        nc.scalar.copy(dq[:], qx1[:])
                        nc.sync.dma_start(dbg["d_qx1"], dq[:])
                        nc.sync.dma_start(dbg["d_red"], red[:])
                        nc.sync.dma_start(dbg["d_s3"], s3[:])
                        dm = npool.tile([128, 3], f32, tag="dm")
                        nc.vector.tensor_copy(dm[:, 0:2], mv[:])
                        nc.vector.tensor_copy(dm[:, 2:3], rs[:])
                        nc.sync.dma_start(dbg["d_mvrs"], dm[:])
                        dxq = npool.tile([128, 528], f32, tag="dxq")
                        nc.vector.tensor_copy(dxq[:], qxs[0][:].bitcast(f32))
                        nc.sync.dma_start(dbg["d_qx"], dxq[:])
                    esw = epool.tile([128, 264], f32, tag="esw")
                    nc.scalar.activation(esw[:, 0:8], s3[:], AF.Exp, scale=SQH)
                    nc.gpsimd.tensor_mul(
                        esw[:, 8:264].rearrange("p (h w) -> p h w", h=H),
                        kvv[:, 256:512].rearrange("p (h w) -> p h w", h=H),
                        esw[:, 0:8].rearrange("p (h one) -> p h one", one=1)
                        .to_broadcast((128, H, HD)))
                    if debug and j == 0:
                        nc.sync.dma_start(dbg["d_esw"], esw[:])
                    nc.tensor.matmul(aggr[:], ohe[:], esw[:],
                                     start=(c_ == 0), stop=(c_ == ch - 1))

                if debug and g == 0:
                    da = npool.tile([128, 264], f32, tag="da")
                    nc.scalar.copy(da[:], aggr[:])
                    nc.sync.dma_start(dbg["d_aggr"], da[:])
                # ---- per-group node phase ----
                den = spool.tile([128, 8], f32, tag="den")
                nc.vector.tensor_scalar(den[:], aggr[:, 0:8], eps16[:], None,
                                        op0=OP.add)
                rec = spool.tile([128, 8], f32, tag="rec")
                nc.vector.reciprocal(rec[:], den[:])
                agn = npool.tile([128, 256], f32, tag="agn")
                nc.vector.tensor_mul(
                    agn[:].rearrange("p (h w) -> p h w", h=H),
                    aggr[:, 8:264].rearrange("p (h w) -> p h w", h=H),
                    rec[:].rearrange("p (h one) -> p h one", one=1)
                    .to_broadcast((128, H, HD)))
                nm = pp.tile([128, 512], f32, tag="nmisc")
                agp = nm[:, 0:256]
                op_ = nm[:, 256:512]
                nc.tensor.transpose(agp[:, 0:128], agn[:, 0:128], ident[:])
                nc.tensor.transpose(agp[:, 128:256], agn[:, 128:256], ident[:])
                agT = npool.tile([128, 256], f32r, tag="agT")
                nc.scalar.copy(agT[:], agp)
                nc.tensor.matmul(op_, agT[:, 0:128], wo_r[:, 0, :],
                                 start=True, stop=False)
                nc.tensor.matmul(op_, agT[:, 128:256], wo_r[:, 1, :],
                                 start=False, stop=False)
                bias_mm(op_, bo_rr[:], stop=True)
                xlt = npool.tile([128, 256], f32, tag="xlt")
                nc.sync.dma_start(xlt[:], xl[g * 128:(g + 1) * 128, :])
                s1 = npool.tile([128, 256], f32, tag="s1")
                nc.vector.tensor_add(s1[:], op_, xlt[:])

                def layer_norm(src_sb, gt, bt, tag):
                    st = spool.tile([128, 6], f32, tag=tag + "st")
                    nc.vector.bn_stats(st[:], src_sb[:])
                    mv_ = spool.tile([128, 2], f32, tag=tag + "mv")
                    nc.vector.bn_aggr(mv_[:], st[:])
                    lv = spool.tile([128, 1], f32, tag=tag + "lv")
                    nc.scalar.activation(lv[:], mv_[:, 1:2], AF.Ln, bias=epst[:])
                    rs_ = spool.tile([128, 1], f32, tag=tag + "rs")
                    nc.scalar.activation(rs_[:], lv[:], AF.Exp, scale=-0.5)
                    xh = npool.tile([128, 256], f32, tag=tag + "xh")
                    nc.vector.scalar_tensor_tensor(
                        xh[:], src_sb[:], mv_[:, 0:1],
                        rs_[:].to_broadcast((128, 256)),
                        op0=OP.subtract, op1=OP.mult)
                    xg = npool.tile([128, 256], f32, tag=tag + "xg")
                    nc.vector.tensor_mul(xg[:], xh[:], gt[:])
                    xo = npool.tile([128, 256], f32, tag=tag + "xo")
                    nc.vector.tensor_add(xo[:], xg[:], bt[:])
                    return xo

                x1 = layer_norm(s1, g1_t, b1_t, "ln1")
                x1p = pp.tile([128, 512], f32, tag="nmisc")
                nc.tensor.transpose(x1p[:, 0:128], x1[:, 0:128], ident[:])
                nc.tensor.transpose(x1p[:, 128:256], x1[:, 128:256], ident[:])
                x1T = npool.tile([128, 256], f32r, tag="x1T")
                nc.scalar.copy(x1T[:], x1p[:, 0:256])

                o2p = pp.tile([128, 256], f32, tag="o2p")
                for fb in range(2):      # ffn hidden in 2 banks of 512
                    fp = pp.tile([128, 512], f32, tag="fp")
                    nc.tensor.matmul(fp[:], x1T[:, 0:128],
                                     wf1_r[:, 0, fb * 512:(fb + 1) * 512],
                                     start=True, stop=False)
                    nc.tensor.matmul(fp[:], x1T[:, 128:256],
                                     wf1_r[:, 1, fb * 512:(fb + 1) * 512],
                                     start=False, stop=False)
                    bias_mm(fp[:], bf1_rr[:, fb * 512:(fb + 1) * 512], stop=True)
                    fsb = npool.tile([128, 512], f32r, tag="fsb")
                    nc.scalar.activation(fsb[:], fp[:], AF.Relu)
                    for q4 in range(4):
                        fqp = pp.tile([128, 128], f32, tag="fqp")
                        nc.tensor.transpose(
                            fqp[:], fsb[:, q4 * 128:(q4 + 1) * 128]
                            .bitcast(f32), ident[:])
                        fqT = npool.tile([128, 128], f32r, tag="fqT")
                        nc.scalar.copy(fqT[:], fqp[:])
                        nc.tensor.matmul(o2p[:], fqT[:],
                                         wf2_r[:, fb * 4 + q4, :],
                                         start=(fb == 0 and q4 == 0),
                                         stop=False)
                bias_mm(o2p[:], bf2_rr[:], stop=True)
                s2s = npool.tile([128, 256], f32, tag="s2s")
                nc.vector.tensor_add(s2s[:], o2p[:], x1[:])
                res = layer_norm(s2s, g2_t, b2_t, "ln2")
                nc.sync.dma_start(out_d[g * 128:(g + 1) * 128, :], res[:])

    nc.compile()
    return nc


# ----------------------------------------------------------------------
# host wrapper
# ----------------------------------------------------------------------
_CACHE = {}


def _get_nc(ch, nch, ncalls, reps=1, debug=False):
    key = (ch, nch, ncalls, reps, debug)
    if key not in _CACHE:
        _CACHE[key] = _build(ch, nch, ncalls, reps, debug)
    return _CACHE[key]


def make_in_maps(x, pos, edge_index, Wq, bq, Wk, bk, Wv, bv, Wp1, bp1, Wp2,
                 bp2, gp, bp, Wo, bo, g1, b1n, Wf1, bf1, Wf2, bf2, g2, b2n):
    f = np.float32
    x = np.asarray(x, f)
    pos = np.asarray(pos, f)
    per_core, padg, ch, nch, ncalls = _prep(edge_index)

    xp = np.zeros((NPAD, D), f)
    xp[:N] = x
    posp = np.zeros((NPAD, 4), f)
    posp[:N, :3] = pos
    xT_full = np.ascontiguousarray(xp.T)

    rep = lambda v: np.broadcast_to(np.asarray(v, f)[None, :],
                                    (128, len(v))).copy()
    row = lambda v: np.asarray(v, f)[None, :].copy()
    shared = {
        "xT": xT_full, "posf": posp,
        "wkv": np.concatenate([np.asarray(Wk, f), np.asarray(Wv, f)], 1),
        "bkv": row(np.concatenate([np.asarray(bk, f) + np.asarray(bp, f),
                                   np.asarray(bv, f)])),
        "wq": np.asarray(Wq, f), "bq_r": row(bq),
        "gp_b": rep(gp), "bp_b": rep(bp),
        "wp1": np.concatenate([np.asarray(Wp1, f), np.zeros((1, 256), f)], 0),
        "bp1c": np.asarray(bp1, f)[:, None].copy(),
        "wp2": np.asarray(Wp2, f), "bp2_r": row(bp2),
        "wo": np.asarray(Wo, f), "bo_r": row(bo),
        "g1_b": rep(g1), "b1_b": rep(b1n),
        "wf1": np.asarray(Wf1, f), "bf1_r": row(bf1),
        "wf2": np.asarray(Wf2, f), "bf2_r": row(bf2),
        "g2_b": rep(g2), "b2_b": rep(b2n),
    }
    in_maps = []
    for c in range(NCORES):
        kvix, drel = per_core[c]
        m = dict(shared)
        m["xTl"] = np.ascontiguousarray(xp[c * NPC:(c + 1) * NPC].T)
        m["xl"] = xp[c * NPC:(c + 1) * NPC].copy()
        m["posl"] = posp[c * NPC:(c + 1) * NPC].copy()
        m["kvidx"] = kvix
        m["dstrel"] = drel
        in_maps.append(m)
    return in_maps, ch, nch, ncalls


def kernel(x, pos, edge_index, Wq, bq, Wk, bk, Wv, bv, Wp1, bp1, Wp2, bp2,
           gp, bp, Wo, bo, g1, b1n, Wf1, bf1, Wf2, bf2, g2, b2n,
           _reps=1, _return_results=False):
    in_maps, ch, nch, ncalls = make_in_maps(
        x, pos, edge_index, Wq, bq, Wk, bk, Wv, bv, Wp1, bp1, Wp2, bp2,
        gp, bp, Wo, bo, g1, b1n, Wf1, bf1, Wf2, bf2, g2, b2n)
    nc = _get_nc(ch, nch, ncalls, _reps)
    res = run_bass_kernel_spmd(nc, in_maps, list(range(NCORES)))
    out = np.concatenate([res.results[c]["out"] for c in range(NCORES)], 0)
    if _return_results:
        return out[:N], res
    return out[:N]


# revision 23
# speedup vs baseline: 1.1787x; 1.1787x over previous
"""ASGFormer (graph-transformer message passing) Trainium2 kernel.

Strategy (8 NeuronCores, SPMD):
  * Nodes are sharded by destination: core c owns nodes [c*1280, (c+1)*1280)
    (N=10000 padded to 10240). Edges are sorted by dst on the host so every
    edge lands on the core that owns its target node; the segment softmax and
    the scatter-add aggregation become core-local (no collectives at all).
  * Per 128-node group, edges are processed in 128-edge chunks. A 0/1
    one-hot [edge, local-node] matrix (built with iota + is_equal) turns both
    the "expand Q to edges" gather and the "scatter-add to nodes" reduction
    into TensorEngine matmuls that accumulate in PSUM.
  * K|V|pos rows are gathered per edge from a precomputed DRAM table with
    the hardware row-gather DMA (dma_gather).
  * The softmax max-subtraction is dropped (mathematically a no-op for
    softmax; scores here are O(1) so exp cannot overflow), and the
    positional-embedding LayerNorm is folded into the score:
        q . (k + LN(y)*g + b) = q.k + qb + rs*( (q*g).y - mu*sum(q*g) )
    with qb/qgs precomputed per node, so pe is never materialized.
  * fp32 everywhere except the bulk per-edge matmuls which use float32r
    (TF32-like, ~1e-4 relative) for 4x TensorEngine throughput. The final
    edge->node aggregation matmul stays fp32.
"""
import numpy as np
from contextlib import ExitStack

import concourse.bass as bass
import concourse.tile as tile
from concourse import bacc, mybir
from concourse.bass_utils import run_bass_kernel_spmd

# Pin all our activation functions (Copy/Identity/Relu/Ln/Exp) to the one
# table set that contains them all (natural_log_exp_and_others) so the
# compiler never inserts per-chunk ACT table reloads (~2.7us each).
import concourse.bacc as _bacc_mod
_orig_get_tables = _bacc_mod.get_activation_tables
def _pinned_tables(arch):
    tabs = _orig_get_tables(arch)
    mine = {mybir.ActivationFunctionType.Exp, mybir.ActivationFunctionType.Ln,
            mybir.ActivationFunctionType.Relu, mybir.ActivationFunctionType.Copy,
            mybir.ActivationFunctionType.Identity}
    out = {}
    for name, fns in tabs.items():
        if name == "natural_log_exp_and_others":
            out[name] = fns
        else:
            out[name] = fns - mine
    return out
_bacc_mod.get_activation_tables = _pinned_tables

f32 = mybir.dt.float32
f32r = mybir.dt.float32r
i16 = mybir.dt.int16
i32 = mybir.dt.int32
AF = mybir.ActivationFunctionType
OP = mybir.AluOpType

N, E, D, H = 10000, 160000, 256, 8
HD = D // H
NCORES = 8
NPC = 1280            # nodes per core (padded)
GPC = 10              # 128-node groups per core
NPAD = NCORES * NPC   # 10240
GCALL = 8             # chunks per kv-gather call
KVW = 576             # kv row: K(256) | V(256) | pos(3) | pad -> 2304B


# ----------------------------------------------------------------------
# host-side index prep (pure index manipulation, no float math)
# ----------------------------------------------------------------------
def _prep(edge_index):
    src = np.asarray(edge_index[0]).astype(np.int64)
    dst = np.asarray(edge_index[1]).astype(np.int64)
    order = np.argsort(dst, kind="stable")
    src_s, dst_s = src[order], dst[order]
    gid = dst_s // 128
    counts = np.bincount(gid, minlength=NCORES * GPC)
    padg = int(np.ceil(max(counts.max(), 1) / 128) * 128)
    ch = padg // 128                       # chunks per group
    nch = GPC * ch                         # chunks per core
    ncalls = int(np.ceil(nch / GCALL))
    nch_pad = ncalls * GCALL
    pos_ = np.concatenate([[0], np.cumsum(counts)])
    per_core = []
    for c in range(NCORES):
        srcs = np.zeros(nch_pad * 128, np.int64)
        dstr = -np.ones(nch_pad * 128, np.float32)
        for g in range(GPC):
            gg = c * GPC + g
            s, e = pos_[gg], pos_[gg + 1]
            srcs[g * padg:g * padg + (e - s)] = src_s[s:e]
            dstr[g * padg:g * padg + (e - s)] = dst_s[s:e] - gg * 128
        # kv gather indices, wrapped over 16 partitions: idx j -> [j%16, j//16]
        wrapped = srcs.reshape(-1, 16).T.astype(np.int16)
        kvidx = np.tile(wrapped, (8, 1))
        # dstrel per chunk column: [lane, chunk]
        dstrel = dstr.reshape(nch_pad, 128).T.copy()
        per_core.append((kvidx, dstrel))
    return per_core, padg, ch, nch, ncalls


# ----------------------------------------------------------------------
# device program
# ----------------------------------------------------------------------
def _build(ch, nch, ncalls, reps=1, debug=False):
    nch_pad = ncalls * GCALL
    nc = bacc.Bacc("TRN2", target_bir_lowering=False, debug=False,
                   num_devices=NCORES)
    DT = lambda n, s, d=f32, k="ExternalInput": nc.dram_tensor(n, s, d, kind=k).ap()

    xT = DT("xT", [256, NPAD])            # full x, transposed (replicated)
    posf = DT("posf", [NPAD, 4])          # full pos padded (replicated)
    xTl = DT("xTl", [256, NPC])           # local x transposed
    xl = DT("xl", [NPC, 256])             # local x (residual)
    posl = DT("posl", [NPC, 4])           # local pos padded
    kvidx = DT("kvidx", [128, nch_pad * 8], i16)
    dstrel = DT("dstrel", [128, nch_pad])
    wkv = DT("wkv", [256, 512])           # [Wk | Wv]
    bkv = DT("bkv", [1, 512])             # [bk + bp_beta | bv]
    wq = DT("wq", [256, 256])
    bq_r = DT("bq_r", [1, 256])
    gp_b = DT("gp_b", [128, 256])         # gamma_p broadcast
    bp_b = DT("bp_b", [128, 256])         # beta_p broadcast
    wp1 = DT("wp1", [4, 256])
    wp2 = DT("wp2", [256, 256])
    bp2_r = DT("bp2_r", [1, 256])
    wo = DT("wo", [256, 256])
    bo_r = DT("bo_r", [1, 256])
    g1_b = DT("g1_b", [128, 256])
    b1_b = DT("b1_b", [128, 256])
    wf1 = DT("wf1", [256, 1024])
    bf1_r = DT("bf1_r", [1, 1024])
    wf2 = DT("wf2", [1024, 256])
    bf2_r = DT("bf2_r", [1, 256])
    g2_b = DT("g2_b", [128, 256])
    b2_b = DT("b2_b", [128, 256])
    out_d = DT("out", [NPC, 256], f32, "ExternalOutput")
    dbg = {}
    if debug:
        for nm, sh in [("d_kvv", [128, KVW]), ("d_ohe", [128, 128]),
                       ("d_dpt", [4, 128]), ("d_h1s", [128, 256]),
                       ("d_ysb", [128, 256]), ("d_qx1", [128, 512]),
                       ("d_red", [128, 16]), ("d_s3", [128, 8]),
                       ("d_esw", [128, 264]), ("d_aggr", [128, 264]),
                       ("d_mvrs", [128, 3]), ("d_qx", [128, 528])]:
            dbg[nm] = DT(nm, sh, f32, "ExternalOutput")
    kv_dram = nc.dram_tensor("kv_dram", [NPAD, KVW], f32, kind="Internal").ap()

    SQH = float(1.0 / np.sqrt(HD))

    with tile.TileContext(nc) as tc, ExitStack() as ctx:
        cpool = ctx.enter_context(tc.tile_pool(name="consts", bufs=1))
        wpool = ctx.enter_context(tc.tile_pool(name="weights", bufs=1))
        qxpool = ctx.enter_context(tc.tile_pool(name="qx", bufs=1))
        xpool = ctx.enter_context(tc.tile_pool(name="xslab", bufs=2))
        kvpool = ctx.enter_context(tc.tile_pool(name="kvg", bufs=3))
        ohpool = ctx.enter_context(tc.tile_pool(name="ohe", bufs=24))
        stpool = ctx.enter_context(tc.tile_pool(name="stage", bufs=2))
        epool = ctx.enter_context(tc.tile_pool(name="edge", bufs=2))
        spool = ctx.enter_context(tc.tile_pool(name="small", bufs=2))
        npool = ctx.enter_context(tc.tile_pool(name="node", bufs=1))
        # PSUM: exactly 8 banks, all single-buffered
        pp = ctx.enter_context(tc.tile_pool(name="ps", bufs=1, space="PSUM"))

        # ---- constants ----
        iota_i = cpool.tile([128, 128], i32)
        nc.gpsimd.iota(iota_i[:], pattern=[[1, 128]], base=0, channel_multiplier=0)
        iota_f = cpool.tile([128, 128], f32)
        nc.vector.tensor_copy(iota_f[:], iota_i[:])
        pidx_i = cpool.tile([128, 1], i32)
        nc.gpsimd.iota(pidx_i[:], pattern=[[0, 1]], base=0, channel_multiplier=1)
        pidx_f = cpool.tile([128, 1], f32)
        nc.vector.tensor_copy(pidx_f[:], pidx_i[:])
        ident = cpool.tile([128, 128], f32)
        nc.vector.tensor_scalar(ident[:], iota_f[:], pidx_f[:], None, op0=OP.is_equal)
        ones1f = cpool.tile([1, 128], f32)
        nc.vector.memset(ones1f[:], 1.0)
        ones1r = cpool.tile([1, 128], f32r)
        nc.vector.tensor_copy(ones1r[:], ones1f[:])
        epst = cpool.tile([128, 1], f32)
        nc.vector.memset(epst[:], 1e-5)
        eps16 = cpool.tile([128, 1], f32)
        nc.vector.memset(eps16[:], 1e-16)
        zpad = cpool.tile([128, 64], f32)
        nc.vector.memset(zpad[:], 0.0)

        # ---- weights: load (+ cast to f32r where used in f32r matmuls) ----
        def ldw(name, dram_ap, shape, dt):
            t = wpool.tile(shape, dt, tag=name)
            nc.gpsimd.dma_start(t[:], dram_ap)
            return t
        wkv_r = ldw("wkv_r", wkv.rearrange("(k p) c -> p k c", p=128),
                    [128, 2, 512], f32r)
        bkv_r = ldw("bkv_r", bkv, [1, 512], f32r)
        wq_r = ldw("wq_r", wq.rearrange("(k p) c -> p k c", p=128),
                   [128, 2, 256], f32r)
        bq_rr = ldw("bq_rr", bq_r, [1, 256], f32r)
        wp1_f = ldw("wp1_f", wp1, [4, 256], f32)
        wp1_r = cpool.tile([4, 256], f32r)
        nc.vector.tensor_scalar_mul(wp1_r[:], wp1_f[:], -1.0)
        wp2_r = ldw("wp2_r", wp2.rearrange("(k p) c -> p k c", p=128),
                    [128, 2, 256], f32r)
        bp2_rr = ldw("bp2_rr", bp2_r, [1, 256], f32r)
        wo_r = ldw("wo_r", wo.rearrange("(k p) c -> p k c", p=128),
                   [128, 2, 256], f32r)
        bo_rr = ldw("bo_rr", bo_r, [1, 256], f32r)
        wf1_r = ldw("wf1_r", wf1.rearrange("(k p) c -> p k c", p=128),
                    [128, 2, 1024], f32r)
        bf1_rr = ldw("bf1_rr", bf1_r, [1, 1024], f32r)
        wf2_r = ldw("wf2_r", wf2.rearrange("(s p) c -> p s c", p=128),
                    [128, 8, 256], f32r)
        bf2_rr = ldw("bf2_rr", bf2_r, [1, 256], f32r)
        gp_t = ldw("gp_t", gp_b, [128, 256], f32)
        bp_t = ldw("bp_t", bp_b, [128, 256], f32)
        g1_t = ldw("g1_t", g1_b, [128, 256], f32)
        b1_t = ldw("b1_t", b1_b, [128, 256], f32)
        g2_t = ldw("g2_t", g2_b, [128, 256], f32)
        b2_t = ldw("b2_t", b2_b, [128, 256], f32)
        kvix = ldw("kvix", kvidx, [128, nch_pad * 8], i16)
        drel = ldw("drel", dstrel, [128, nch_pad], f32)
        posl_f = ldw("posl_f", posl.rearrange("(g p) c -> p g c", p=128),
                     [128, GPC, 4], f32)
        posl_r = cpool.tile([128, GPC, 4], f32r)
        nc.vector.tensor_scalar_mul(posl_r[:], posl_f[:], -1.0)

        def bias_mm(ps_ap, row_r, start=False, stop=False):
            nc.tensor.matmul(ps_ap, ones1r[:, 0:ps_ap.shape[0]], row_r,
                             start=start, stop=stop)

        with nc.allow_low_precision("f32r intermediate stages by design"):
          for rep in range(reps):
            # ============ phase 1: KV table (replicated full graph) ============
            for sl in range(16):
                xs0 = xpool.tile([128, 640], f32r, tag="xs0")
                nc.gpsimd.dma_start(xs0[:], xT[0:128, sl * 640:(sl + 1) * 640])
                xs1 = xpool.tile([128, 640], f32r, tag="xs1")
                nc.gpsimd.dma_start(xs1[:], xT[128:256, sl * 640:(sl + 1) * 640])
                for t in range(5):
                    nt = sl * 5 + t
                    kvp = pp.tile([128, 512], f32, tag="bank_a")
                    nc.tensor.matmul(kvp[:], xs0[:, t * 128:(t + 1) * 128],
                                     wkv_r[:, 0, :], start=True, stop=False)
                    nc.tensor.matmul(kvp[:], xs1[:, t * 128:(t + 1) * 128],
                                     wkv_r[:, 1, :], start=False, stop=False)
                    bias_mm(kvp[:], bkv_r[:], stop=True)
                    kvs = npool.tile([128, 512], f32, tag="kvs")
                    nc.scalar.copy(kvs[:], kvp[:])
                    nc.sync.dma_start(
                        kv_dram[nt * 128:(nt + 1) * 128, 0:512], kvs[:])
                    nc.sync.dma_start(
                        kv_dram[nt * 128:(nt + 1) * 128, 515:576],
                        zpad[:, 0:61])
            nc.sync.dma_start(kv_dram[:, 512:515], posf[:, 0:3])

            # ============ phase 2: Q-side per group ============
            xlT = wpool.tile([128, 2, NPC], f32r, tag="xlT")
            nc.gpsimd.dma_start(xlT[:], xTl.rearrange("(k p) c -> p k c", p=128))
            qxs = []
            for g in range(GPC):
                qp = pp.tile([128, 256], f32, tag="bank_a")
                nc.tensor.matmul(qp[:], xlT[:, 0, g * 128:(g + 1) * 128],
                                 wq_r[:, 0, :], start=True, stop=False)
                nc.tensor.matmul(qp[:], xlT[:, 1, g * 128:(g + 1) * 128],
                                 wq_r[:, 1, :], start=False, stop=False)
                bias_mm(qp[:], bq_rr[:], stop=True)
                qx = qxpool.tile([128, 528], f32r, tag=f"qx{g}")
                nc.scalar.copy(qx[:, 0:256], qp[:])
                nc.vector.tensor_mul(qx[:, 256:512], qp[:], gp_t[:])
                qbt = npool.tile([128, 256], f32, tag="qbt")
                nc.vector.tensor_mul(qbt[:], qp[:], bp_t[:])
                nc.vector.tensor_reduce(
                    qx[:, 512:520].rearrange("p (h one) -> p h one", one=1),
                    qbt[:].rearrange("p (h w) -> p h w", h=H),
                    axis=mybir.AxisListType.X, op=OP.add)
                nc.vector.tensor_reduce(
                    qx[:, 520:528].rearrange("p (h one) -> p h one", one=1),
                    qx[:, 256:512].rearrange("p (h w) -> p h w", h=H),
                    axis=mybir.AxisListType.X, op=OP.add)
                qxs.append(qx)

            # ============ phase 3: edge chunks ============
            # PSUM banks: bank_a(ohp|dpp), h1p, yq, qx1, aggr,
            #             nmisc(x3: agp+op_ / x1p / o2p), fp, fqp -> 8
            kvts = {}
            for g in range(GPC):
                aggr = pp.tile([128, 264], f32, tag="aggr")
                st6_st = stpool.tile([128, ch, 6], f32, tag="st6s")
                red_st = stpool.tile([128, ch, 16], f32, tag="reds")
                qbgs_st = stpool.tile([128, ch, 16], f32, tag="qbgss")
                ohes = []
                kvvs = []
                npairs = (ch + 1) // 2
                for pr in range(npairs):
                    c0 = pr * 2
                    pw = min(2, ch - c0)          # pair width (1 or 2)
                    pwc = pw * 128
                    # --- gather + onehot for each chunk of the pair ---
                    bank_a = pp.tile([128, 512], f32, tag="bank_a")
                    ohp = bank_a[:, 0:256]
                    dpp = bank_a[0:4, 256:512]
                    ohn = epool.tile([128, 256], f32r, tag="ohn")
                    pair_kvv = []
                    for ci in range(pw):
                        c_ = c0 + ci
                        j = g * ch + c_
                        call = j // GCALL
                        if call not in kvts:
                            kvt = kvpool.tile([128, GCALL, KVW], f32, tag="kvt")
                            nc.gpsimd.dma_gather(
                                kvt[:], kv_dram[:, :],
                                kvix[:, call * GCALL * 8:(call + 1) * GCALL * 8],
                                num_idxs=GCALL * 128, num_idxs_reg=GCALL * 128,
                                elem_size=KVW)
                            kvts = {call: kvt}
                        kvv = kvts[call][:, j % GCALL, :]
                        pair_kvv.append(kvv)
                        kvvs.append(kvv)
                        ohe = ohpool.tile([128, 128], f32, tag="ohe")
                        nc.vector.tensor_scalar(ohe[:], iota_f[:],
                                                drel[:, j:j + 1], None,
                                                op0=OP.is_equal)
                        ohes.append(ohe)
                        nc.tensor.matmul(ohp[:, ci * 128:(ci + 1) * 128],
                                         ohe[:], ident[:], is_transpose=True,
                                         start=True, stop=True)
                    nc.scalar.copy(ohn[:, 0:pwc], ohp[:, 0:pwc])
                    # --- dp^T (negated) for the pair: transpose(pos_s) - pos_d
                    for ci in range(pw):
                        nc.tensor.matmul(
                            dpp[0:4, ci * 128:(ci + 1) * 128],
                            pair_kvv[ci][:, 512:516], ident[:],
                            is_transpose=True, start=(ci == 0), stop=False)
                    nc.tensor.matmul(dpp[0:4, 0:pwc], posl_r[:, g, :],
                                     ohn[:, 0:pwc], start=False, stop=True)
                    dpt = spool.tile([4, 256], f32r, tag="dpt")
                    nc.scalar.copy(dpt[:, 0:pwc], dpp[0:4, 0:pwc])
                    # --- h1 for the pair (bias folded via wp1_r row 3) ---
                    h1p = pp.tile([128, 512], f32, tag="h1p")
                    nc.tensor.matmul(h1p[:, 0:pwc], wp1_r[:, 0:128],
                                     dpt[:, 0:pwc], start=True, stop=True)
                    nc.tensor.matmul(h1p[:, 256:256 + pwc], wp1_r[:, 128:256],
                                     dpt[:, 0:pwc], start=True, stop=True)
                    h1s = epool.tile([128, 512], f32r, tag="h1s")
                    if pw == 2:
                        nc.scalar.activation(h1s[:], h1p[:], AF.Relu)
                    else:
                        nc.scalar.activation(h1s[:, 0:128], h1p[:, 0:128],
                                             AF.Relu)
                        nc.scalar.activation(h1s[:, 256:384], h1p[:, 256:384],
                                             AF.Relu)
                    # --- per chunk: y, qx, scores prep ---
                    for ci in range(pw):
                        c_ = c0 + ci
                        kvv = pair_kvv[ci]
                        ohn_c = ohn[:, ci * 128:(ci + 1) * 128]
                        yq = pp.tile([128, 272], f32, tag="yq")
                        y = yq[:, 0:256]
                        nc.tensor.matmul(y, h1s[:, ci * 128:ci * 128 + 128],
                                         wp2_r[:, 0, :], start=True, stop=False)
                        nc.tensor.matmul(y, h1s[:, 256 + ci * 128:384 + ci * 128],
                                         wp2_r[:, 1, :], start=False, stop=False)
                        bias_mm(y, bp2_rr[:], stop=True)
                        nc.tensor.matmul(yq[:, 256:272], ohn_c,
                                         qxs[g][:, 512:528], start=True,
                                         stop=True)
                        qx1 = pp.tile([128, 512], f32, tag="qx1")
                        nc.tensor.matmul(qx1[:], ohn_c, qxs[g][:, 0:512],
                                         start=True, stop=True)
                        qall = epool.tile([128, 512], f32, tag="qall")
                        nc.scalar.copy(qall[:], qx1[:])
                        nc.scalar.copy(qbgs_st[:, c_, :], yq[:, 256:272])
                        nc.vector.bn_stats(st6_st[:, c_, :], y)
                        u2 = epool.tile([128, 512], f32, tag="u2")
                        nc.gpsimd.tensor_mul(u2[:, 0:256], qall[:, 0:256],
                                             kvv[:, 0:256])
                        nc.vector.tensor_mul(u2[:, 256:512], qall[:, 256:512],
                                             y)
                        nc.vector.tensor_reduce(
                            red_st[:, c_, :].rearrange(
                                "p (h one) -> p h one", one=1),
                            u2[:].rearrange("p (h w) -> p h w", h=16),
                            axis=mybir.AxisListType.X, op=OP.add)

                # --- batched per-group: stats combine, rs, scores, exp ---
                me = st6_st[:, :, 1]
                mo = st6_st[:, :, 4]
                cve = st6_st[:, :, 2]
                cvo = st6_st[:, :, 5]
                t1 = stpool.tile([128, ch], f32, tag="bt1")
                nc.vector.tensor_add(t1[:], me, mo)
                mu_t = stpool.tile([128, ch], f32, tag="bmu")
                nc.vector.tensor_scalar_mul(mu_t[:], t1[:], 0.5)
                d1_ = stpool.tile([128, ch], f32, tag="bd1")
                nc.vector.tensor_sub(d1_[:], me, mo)
                d2_ = stpool.tile([128, ch], f32, tag="bd2")
                nc.vector.tensor_mul(d2_[:], d1_[:], d1_[:])
                cvs = stpool.tile([128, ch], f32, tag="bcvs")
                nc.vector.tensor_add(cvs[:], cve, cvo)
                var_t = stpool.tile([128, ch], f32, tag="bvar")
                nc.vector.scalar_tensor_tensor(
                    var_t[:], d2_[:], 256.0 / 4.0, cvs[:],
                    op0=OP.mult, op1=OP.add)
                var2 = stpool.tile([128, ch], f32, tag="bvar2")
                nc.vector.tensor_scalar_mul(var2[:], var_t[:], 1.0 / 256.0)
                lnv = stpool.tile([128, ch], f32, tag="blnv")
                nc.scalar.activation(lnv[:], var2[:], AF.Ln, bias=epst[:])
                rs_t = stpool.tile([128, ch], f32, tag="brs")
                nc.scalar.activation(rs_t[:], lnv[:], AF.Exp, scale=-0.5)
                qk3 = red_st[:, :, 0:8]
                qgy3 = red_st[:, :, 8:16]
                qb3 = qbgs_st[:, :, 0:8]
                qgs3 = qbgs_st[:, :, 8:16]
                mu_b = mu_t[:].rearrange("p (c one) -> p c one", one=1)                     .to_broadcast((128, ch, 8))
                rs_b = rs_t[:].rearrange("p (c one) -> p c one", one=1)                     .to_broadcast((128, ch, 8))
                m1 = stpool.tile([128, ch, 8], f32, tag="bm1")
                nc.vector.tensor_mul(m1[:], qgs3, mu_b)
                m2 = stpool.tile([128, ch, 8], f32, tag="bm2")
                nc.vector.tensor_sub(m2[:], qgy3, m1[:])
                m3 = stpool.tile([128, ch, 8], f32, tag="bm3")
                nc.vector.tensor_mul(m3[:], m2[:], rs_b)
                a1 = stpool.tile([128, ch, 8], f32, tag="ba1")
                nc.vector.tensor_add(a1[:], qk3, qb3)
                a2 = stpool.tile([128, ch, 8], f32, tag="ba2")
                nc.vector.tensor_add(a2[:], a1[:], m3[:])
                es_st = stpool.tile([128, ch, 8], f32, tag="bes")
                nc.scalar.activation(
                    es_st[:].rearrange("p c h -> p (c h)"),
                    a2[:].rearrange("p c h -> p (c h)"), AF.Exp, scale=SQH)

                # --- pass B: weighted messages + aggregation matmuls ---
                for c_ in range(ch):
                    kvv = kvvs[c_]
                    w_t = epool.tile([128, 256], f32, tag="w_t")
                    nc.gpsimd.tensor_mul(
                        w_t[:].rearrange("p (h w) -> p h w", h=H),
                        kvv[:, 256:512].rearrange("p (h w) -> p h w", h=H),
                        es_st[:, c_, :].rearrange("p (h one) -> p h one",
                                                  one=1)
                        .to_broadcast((128, H, HD)))
                    nc.tensor.matmul(aggr[:, 0:8], ohes[c_][:],
                                     es_st[:, c_, :],
                                     start=(c_ == 0), stop=False)
                    nc.tensor.matmul(aggr[:, 8:264], ohes[c_][:], w_t[:],
                                     start=False, stop=(c_ == ch - 1))

                # ---- per-group node phase ----
                den = spool.tile([128, 8], f32, tag="den")
                nc.vector.tensor_scalar(den[:], aggr[:, 0:8], eps16[:], None,
                                        op0=OP.add)
                rec = spool.tile([128, 8], f32, tag="rec")
                nc.vector.reciprocal(rec[:], den[:])
                agn = npool.tile([128, 256], f32, tag="agn")
                nc.vector.tensor_mul(
                    agn[:].rearrange("p (h w) -> p h w", h=H),
                    aggr[:, 8:264].rearrange("p (h w) -> p h w", h=H),
                    rec[:].rearrange("p (h one) -> p h one", one=1)
                    .to_broadcast((128, H, HD)))
                nm = pp.tile([128, 512], f32, tag="nmisc")
                agp = nm[:, 0:256]
                op_ = nm[:, 256:512]
                nc.tensor.transpose(agp[:, 0:128], agn[:, 0:128], ident[:])
                nc.tensor.transpose(agp[:, 128:256], agn[:, 128:256], ident[:])
                agT = npool.tile([128, 256], f32r, tag="agT")
                nc.scalar.copy(agT[:], agp)
                nc.tensor.matmul(op_, agT[:, 0:128], wo_r[:, 0, :],
                                 start=True, stop=False)
                nc.tensor.matmul(op_, agT[:, 128:256], wo_r[:, 1, :],
                                 start=False, stop=False)
                bias_mm(op_, bo_rr[:], stop=True)
                xlt = npool.tile([128, 256], f32, tag="xlt")
                nc.sync.dma_start(xlt[:], xl[g * 128:(g + 1) * 128, :])
                s1 = npool.tile([128, 256], f32, tag="s1")
                nc.vector.tensor_add(s1[:], op_, xlt[:])

                def layer_norm(src_sb, gt, bt, tag):
                    st = spool.tile([128, 6], f32, tag=tag + "st")
                    nc.vector.bn_stats(st[:], src_sb[:])
                    mv_ = spool.tile([128, 2], f32, tag=tag + "mv")
                    nc.vector.bn_aggr(mv_[:], st[:])
                    lv = spool.tile([128, 1], f32, tag=tag + "lv")
                    nc.scalar.activation(lv[:], mv_[:, 1:2], AF.Ln, bias=epst[:])
                    rs_ = spool.tile([128, 1], f32, tag=tag + "rs")
                    nc.scalar.activation(rs_[:], lv[:], AF.Exp, scale=-0.5)
                    xh = npool.tile([128, 256], f32, tag=tag + "xh")
                    nc.vector.scalar_tensor_tensor(
                        xh[:], src_sb[:], mv_[:, 0:1],
                        rs_[:].to_broadcast((128, 256)),
                        op0=OP.subtract, op1=OP.mult)
                    xg = npool.tile([128, 256], f32, tag=tag + "xg")
                    nc.vector.tensor_mul(xg[:], xh[:], gt[:])
                    xo = npool.tile([128, 256], f32, tag=tag + "xo")
                    nc.vector.tensor_add(xo[:], xg[:], bt[:])
                    return xo

                x1 = layer_norm(s1, g1_t, b1_t, "ln1")
                x1p = pp.tile([128, 512], f32, tag="nmisc")
                nc.tensor.transpose(x1p[:, 0:128], x1[:, 0:128], ident[:])
                nc.tensor.transpose(x1p[:, 128:256], x1[:, 128:256], ident[:])
                x1T = npool.tile([128, 256], f32r, tag="x1T")
                nc.scalar.copy(x1T[:], x1p[:, 0:256])

                o2t = pp.tile([128, 512], f32, tag="nmisc")
                o2p = o2t[:, 0:256]
                for fb in range(2):      # ffn hidden in 2 banks of 512
                    fp = pp.tile([128, 512], f32, tag="fp")
                    nc.tensor.matmul(fp[:], x1T[:, 0:128],
                                     wf1_r[:, 0, fb * 512:(fb + 1) * 512],
                                     start=True, stop=False)
                    nc.tensor.matmul(fp[:], x1T[:, 128:256],
                                     wf1_r[:, 1, fb * 512:(fb + 1) * 512],
                                     start=False, stop=False)
                    bias_mm(fp[:], bf1_rr[:, fb * 512:(fb + 1) * 512], stop=True)
                    fsb = npool.tile([128, 512], f32r, tag="fsb")
                    nc.scalar.activation(fsb[:], fp[:], AF.Relu)
                    for q4 in range(4):
                        fqp = pp.tile([128, 128], f32, tag="fqp")
                        nc.tensor.transpose(
                            fqp[:], fsb[:, q4 * 128:(q4 + 1) * 128]
                            .bitcast(f32), ident[:])
                        fqT = npool.tile([128, 128], f32r, tag="fqT")
                        nc.scalar.copy(fqT[:], fqp[:])
                        nc.tensor.matmul(o2p, fqT[:],
                                         wf2_r[:, fb * 4 + q4, :],
                                         start=(fb == 0 and q4 == 0),
                                         stop=False)
                bias_mm(o2p, bf2_rr[:], stop=True)
                s2s = npool.tile([128, 256], f32, tag="s2s")
                nc.vector.tensor_add(s2s[:], o2p, x1[:])
                res = layer_norm(s2s, g2_t, b2_t, "ln2")
                nc.sync.dma_start(out_d[g * 128:(g + 1) * 128, :], res[:])

    nc.compile()
    return nc


# ----------------------------------------------------------------------
# host wrapper
# ----------------------------------------------------------------------
_CACHE = {}


def _get_nc(ch, nch, ncalls, reps=1, debug=False):
    key = (ch, nch, ncalls, reps, debug)
    if key not in _CACHE:
        _CACHE[key] = _build(ch, nch, ncalls, reps, debug)
    return _CACHE[key]


def make_in_maps(x, pos, edge_index, Wq, bq, Wk, bk, Wv, bv, Wp1, bp1, Wp2,
                 bp2, gp, bp, Wo, bo, g1, b1n, Wf1, bf1, Wf2, bf2, g2, b2n):
    f = np.float32
    x = np.asarray(x, f)
    pos = np.asarray(pos, f)
    per_core, padg, ch, nch, ncalls = _prep(edge_index)

    xp = np.zeros((NPAD, D), f)
    xp[:N] = x
    posp = np.zeros((NPAD, 4), f)
    posp[:N, :3] = pos
    posp[:, 3] = 1.0
    xT_full = np.ascontiguousarray(xp.T)

    rep = lambda v: np.broadcast_to(np.asarray(v, f)[None, :],
                                    (128, len(v))).copy()
    row = lambda v: np.asarray(v, f)[None, :].copy()
    shared = {
        "xT": xT_full, "posf": posp,
        "wkv": np.concatenate([np.asarray(Wk, f), np.asarray(Wv, f)], 1),
        "bkv": row(np.concatenate([np.asarray(bk, f) + np.asarray(bp, f),
                                   np.asarray(bv, f)])),
        "wq": np.asarray(Wq, f), "bq_r": row(bq),
        "gp_b": rep(gp), "bp_b": rep(bp),
        "wp1": np.concatenate([np.asarray(Wp1, f),
                               np.asarray(bp1, f)[None, :]], 0),
        "wp2": np.asarray(Wp2, f), "bp2_r": row(bp2),
        "wo": np.asarray(Wo, f), "bo_r": row(bo),
        "g1_b": rep(g1), "b1_b": rep(b1n),
        "wf1": np.asarray(Wf1, f), "bf1_r": row(bf1),
        "wf2": np.asarray(Wf2, f), "bf2_r": row(bf2),
        "g2_b": rep(g2), "b2_b": rep(b2n),
    }
    in_maps = []
    for c in range(NCORES):
        kvix, drel = per_core[c]
        m = dict(shared)
        m["xTl"] = np.ascontiguousarray(xp[c * NPC:(c + 1) * NPC].T)
        m["xl"] = xp[c * NPC:(c + 1) * NPC].copy()
        m["posl"] = posp[c * NPC:(c + 1) * NPC].copy()
        m["kvidx"] = kvix
        m["dstrel"] = drel
        in_maps.append(m)
    return in_maps, ch, nch, ncalls


def kernel(x, pos, edge_index, Wq, bq, Wk, bk, Wv, bv, Wp1, bp1, Wp2, bp2,
           gp, bp, Wo, bo, g1, b1n, Wf1, bf1, Wf2, bf2, g2, b2n,
           _reps=1, _return_results=False):
    in_maps, ch, nch, ncalls = make_in_maps(
        x, pos, edge_index, Wq, bq, Wk, bk, Wv, bv, Wp1, bp1, Wp2, bp2,
        gp, bp, Wo, bo, g1, b1n, Wf1, bf1, Wf2, bf2, g2, b2n)
    nc = _get_nc(ch, nch, ncalls, _reps)
    res = run_bass_kernel_spmd(nc, in_maps, list(range(NCORES)))
    out = np.concatenate([res.results[c]["out"] for c in range(NCORES)], 0)
    if _return_results:
        return out[:N], res
    return out[:N]
